# revision 3
# baseline (speedup 1.0000x reference)
"""MoE routing kernel for Trainium2 (8 NeuronCores, Bass/Tile).

Problem: y = relu(x @ w[idxs] + b[idxs])
  x: [8192, 1024] f32, idxs: [8192] int in [0,16), w: [16, 1024, 1024] f32,
  b: [16, 1024] f32  ->  y: [8192, 1024] f32

Strategy (expert-parallel):
  - Host: bucket tokens by expert. Assign 2 experts per core (the 8 largest
    experts in slot 0, the 8 smallest in slot 1) so every core runs the SAME
    program: segment 0 = [0, B0) tokens for expert A, segment 1 = [B0, T) for
    expert B, with per-slot padding to the max expert population.
  - Host pre-transposes each core's token block to x_t [IN_F, T] (fp16) so the
    contraction dim (IN_F) lands on SBUF partitions with contiguous DMAs.
    fp16 (not bf16): same TensorE rate, ~8x lower rounding error (3.6e-4 L2).
  - Device (per core): out-feature tile m is the outer loop; for each m and
    segment, the 8 K-tile matmuls accumulate all of the segment's token
    blocks into per-block PSUM banks (weight tile stays stationary across
    blocks), then one fused ScalarE activation per block does bias-add + ReLU
    from PSUM into an fp16 row buffer [128, T]; one store per m via the
    ScalarE DMA ring (separate from the load ring, no cross-engine stall).
  - Loads are issued in consumption order (x block 0, w quarter 0 of both
    experts, remaining x, remaining w quarters) so compute starts after ~1.5MB
    instead of after the full 6.4MB.
  - Host: transpose back and scatter rows to their original token positions.
"""

import os

import numpy as np

_P = 128
_NCORES = 8
_E = 16
_IN_F = 1024
_OUT_F = 1024
_KT = _IN_F // _P  # 8 contraction tiles
_MT = _OUT_F // _P  # 8 output-feature tiles
_NBLK = 512  # token block (PSUM free dim, fp32 bank limit)
_WQ = 256  # w column-quarter width (512B rows in fp16, DMA line-rate minimum)

# Compute dtype for x/w/y on device: "float32" | "bfloat16" | "float16"
_DTYPE = os.environ.get("MOE_DTYPE", "float16")

_prog_cache: dict = {}
LAST_RESULT = None  # BassKernelResults of the most recent device run


def _blocks(lo, hi):
    out = []
    t = lo
    while t < hi:
        nb = min(_NBLK, hi - t)
        out.append((t, nb))
        t += nb
    return out


def _build_program(T: int, B0: int, dtype_str: str):
    from contextlib import ExitStack

    import concourse.mybir as mybir
    import concourse.tile as tile
    from concourse import bacc

    key = (T, B0, dtype_str)
    if key in _prog_cache:
        return _prog_cache[key]

    data_dt = {
        "float32": mybir.dt.float32,
        "bfloat16": mybir.dt.bfloat16,
        "float16": mybir.dt.float16,
    }[dtype_str]

    nc = bacc.Bacc("TRN2", target_bir_lowering=False, debug=False, num_devices=_NCORES)

    xt_d = nc.dram_tensor("xt", [_IN_F, T], data_dt, kind="ExternalInput")
    w2_d = nc.dram_tensor("w2", [2, _IN_F, _OUT_F], data_dt, kind="ExternalInput")
    bt_d = nc.dram_tensor("bt", [_P, 2 * _MT], mybir.dt.float32, kind="ExternalInput")
    yt_d = nc.dram_tensor("yt", [_OUT_F, T], data_dt, kind="ExternalOutput")

    relu = mybir.ActivationFunctionType.Relu
    segs = [(0, _blocks(0, B0)), (1, _blocks(B0, T))]
    # flat consumption-ordered list of (seg, t0, nb)
    flat_blocks = [(s, t0, nb) for s, blks in segs for t0, nb in blks]
    NQ = _OUT_F // _WQ  # 4 weight quarters per expert

    with tile.TileContext(nc) as tc, ExitStack() as ctx:
        const = ctx.enter_context(tc.tile_pool(name="const", bufs=1))
        xpool = ctx.enter_context(tc.tile_pool(name="xpool", bufs=1))
        wpool = ctx.enter_context(tc.tile_pool(name="wpool", bufs=1))
        ypool = ctx.enter_context(tc.tile_pool(name="ypool", bufs=3))
        pspool = ctx.enter_context(tc.tile_pool(name="pspool", bufs=6, space="PSUM"))

        bt = const.tile([_P, 2 * _MT], mybir.dt.float32, name="bt_sb")
        nc.sync.dma_start(bt[:], bt_d.ap())

        # x token-block tiles: [128, KT, nb], one DMA per k-panel.
        xtiles = {}

        def load_x(bi):
            s, t0, nb = flat_blocks[bi]
            xt = xpool.tile([_P, _KT, nb], data_dt, name=f"x{bi}", tag=f"x{bi}")
            for k in range(_KT):
                nc.sync.dma_start(
                    xt[:, k], xt_d.ap()[k * _P : (k + 1) * _P, t0 : t0 + nb]
                )
            xtiles[bi] = xt

        # w quarter tiles: [128, KT, WQ] per (expert slot, quarter).
        wtiles = {}

        def load_w(s, q):
            wt = wpool.tile([_P, _KT, _WQ], data_dt, name=f"w{s}q{q}", tag=f"w{s}q{q}")
            for k in range(_KT):
                nc.sync.dma_start(
                    wt[:, k],
                    w2_d.ap()[s, k * _P : (k + 1) * _P, q * _WQ : (q + 1) * _WQ],
                )
            wtiles[(s, q)] = wt

        # Consumption-ordered loads: first token block + first weight quarter
        # of both experts, then the rest of x, then remaining quarters.
        load_x(0)
        load_w(0, 0)
        for bi in range(1, len(flat_blocks)):
            load_x(bi)
        load_w(1, 0)
        for q in range(1, NQ):
            load_w(0, q)
            load_w(1, q)

        for m in range(_MT):
            q, qc = divmod(m * _P, _WQ)
            ysb = ypool.tile([_P, T], data_dt, name="ysb", tag="ysb")
            for s, blks in segs:
                wt = wtiles[(s, q)]
                ps = [
                    pspool.tile([_P, _NBLK], mybir.dt.float32, name="ps", tag="ps")[
                        :, :nb
                    ]
                    for _, nb in blks
                ]
                for k in range(_KT):
                    lhsT = wt[:, k, qc : qc + _P]
                    for bi, (t0, nb) in enumerate(blks):
                        bid = flat_blocks.index((s, t0, nb))
                        nc.tensor.matmul(
                            ps[bi],
                            lhsT,
                            xtiles[bid][:, k, :nb],
                            start=(k == 0),
                            stop=(k == _KT - 1),
                        )
                for bi, (t0, nb) in enumerate(blks):
                    nc.scalar.activation(
                        ysb[:, t0 : t0 + nb],
                        ps[bi],
                        relu,
                        bias=bt[:, s * _MT + m : s * _MT + m + 1],
                    )
            # store the finished row block on the ScalarE DMA ring: same
            # engine as the ACTs that produced ysb, so no stall risk.
            nc.scalar.dma_start(yt_d.ap()[m * _P : (m + 1) * _P, :], ysb[:])

    nc.compile()
    _prog_cache[key] = nc
    return nc


def kernel(x, idxs, w, b):
    global LAST_RESULT
    import ml_dtypes
    from concourse.bass_utils import run_bass_kernel_spmd

    x = np.ascontiguousarray(np.asarray(x, dtype=np.float32))
    idxs = np.asarray(idxs).astype(np.int64)
    w = np.ascontiguousarray(np.asarray(w, dtype=np.float32))
    b = np.ascontiguousarray(np.asarray(b, dtype=np.float32))

    n_tok = x.shape[0]
    np_dt = {
        "float32": np.float32,
        "bfloat16": ml_dtypes.bfloat16,
        "float16": np.float16,
    }[_DTYPE]

    counts = np.bincount(idxs, minlength=_E)
    order = np.argsort(-counts, kind="stable")
    slot0 = order[:_NCORES]  # 8 largest experts
    slot1 = order[_E - 1 : _NCORES - 1 : -1]  # 8 smallest, reversed for pairing
    B0 = max(int(counts[slot0].max()), 1)
    B1 = max(int(counts[slot1].max()), 1)
    T = B0 + B1

    tok_of = [np.nonzero(idxs == e)[0] for e in range(_E)]

    in_maps = []
    metas = []
    for c in range(_NCORES):
        eA, eB = int(slot0[c]), int(slot1[c])
        ta, tb = tok_of[eA], tok_of[eB]
        xt = np.zeros((_IN_F, T), dtype=np_dt)
        xt[:, : len(ta)] = x[ta].T.astype(np_dt)
        xt[:, B0 : B0 + len(tb)] = x[tb].T.astype(np_dt)
        w2 = np.stack([w[eA], w[eB]]).astype(np_dt)
        bt = np.empty((_P, 2 * _MT), np.float32)
        bt[:, :_MT] = b[eA].reshape(_MT, _P).T
        bt[:, _MT:] = b[eB].reshape(_MT, _P).T
        in_maps.append({"xt": xt, "w2": w2, "bt": bt})
        metas.append((ta, tb))

    nc = _build_program(T, B0, _DTYPE)

    trace = os.environ.get("MOE_TRACE", "0") == "1"
    kwargs = {}
    if trace:
        kwargs["trace"] = True
        tdir = os.environ.get("MOE_TRACE_DIR")
        if tdir:
            os.makedirs(tdir, exist_ok=True)
            kwargs["tmpdir"] = tdir

    res = run_bass_kernel_spmd(nc, in_maps, core_ids=list(range(_NCORES)), **kwargs)
    LAST_RESULT = res

    y = np.empty((n_tok, _OUT_F), dtype=np.float32)
    for c in range(_NCORES):
        ta, tb = metas[c]
        yt = res.results[c]["yt"].astype(np.float32)
        y[ta] = yt[:, : len(ta)].T
        y[tb] = yt[:, B0 : B0 + len(tb)].T
    return y


# revision 5
# speedup vs baseline: 1.3688x; 1.3688x over previous
"""MoE routing kernel for Trainium2 (8 NeuronCores, Bass/Tile).

Problem: y = relu(x @ w[idxs] + b[idxs])
  x: [8192, 1024] f32, idxs: [8192] int in [0,16), w: [16, 1024, 1024] f32,
  b: [16, 1024] f32  ->  y: [8192, 1024] f32

Strategy (expert-parallel):
  - Host: bucket tokens by expert. Assign 2 experts per core (the 8 largest
    experts in slot 0, the 8 smallest in slot 1) so every core runs the SAME
    program: segment 0 = [0, B0) tokens for expert A, segment 1 = [B0, T) for
    expert B, with per-slot padding to the max expert population.
  - Host pre-transposes each core's token block to x_t [IN_F, T] (fp16) so the
    contraction dim (IN_F) lands on SBUF partitions with contiguous DMAs.
    fp16 (not bf16): same TensorE rate, ~8x lower rounding error (3.6e-4 L2).
  - Device (per core): out-feature tile m is the outer loop; for each m and
    segment, the 8 K-tile matmuls accumulate all of the segment's token
    blocks into per-block PSUM banks (weight tile stays stationary across
    blocks), then one fused ScalarE activation per block does bias-add + ReLU
    from PSUM into an fp16 row buffer [128, T]; one store per m via the
    ScalarE DMA ring (separate from the load ring, no cross-engine stall).
  - Loads are issued in consumption order (x block 0, w quarter 0 of both
    experts, remaining x, remaining w quarters) so compute starts after ~1.5MB
    instead of after the full 6.4MB.
  - Host: transpose back and scatter rows to their original token positions.
"""

import os

import numpy as np

_P = 128
_NCORES = 8
_E = 16
_IN_F = 1024
_OUT_F = 1024
_KT = _IN_F // _P  # 8 contraction tiles
_MT = _OUT_F // _P  # 8 output-feature tiles
_NBLK = 512  # token block (PSUM free dim, fp32 bank limit)
_WQ = 256  # w column-quarter width (512B rows in fp16, DMA line-rate minimum)

# Compute dtype for x/w/y on device: "float32" | "bfloat16" | "float16"
_DTYPE = os.environ.get("MOE_DTYPE", "float16")

_prog_cache: dict = {}
LAST_RESULT = None  # BassKernelResults of the most recent device run


def _blocks(lo, hi, first=None):
    out = []
    t = lo
    if first is not None and hi - lo > first:
        out.append((t, first))
        t += first
    while t < hi:
        nb = min(_NBLK, hi - t)
        out.append((t, nb))
        t += nb
    return out


def _build_program(T: int, B0: int, dtype_str: str):
    from contextlib import ExitStack

    import concourse.mybir as mybir
    import concourse.tile as tile
    from concourse import bacc

    key = (T, B0, dtype_str)
    if key in _prog_cache:
        return _prog_cache[key]

    data_dt = {
        "float32": mybir.dt.float32,
        "bfloat16": mybir.dt.bfloat16,
        "float16": mybir.dt.float16,
    }[dtype_str]

    nc = bacc.Bacc("TRN2", target_bir_lowering=False, debug=False, num_devices=_NCORES)

    xt_d = nc.dram_tensor("xt", [_IN_F, T], data_dt, kind="ExternalInput")
    w2_d = nc.dram_tensor("w2", [2, _IN_F, _OUT_F], data_dt, kind="ExternalInput")
    bt_d = nc.dram_tensor("bt", [_P, 2 * _MT], mybir.dt.float32, kind="ExternalInput")
    yt_d = nc.dram_tensor("yt", [_OUT_F, T], data_dt, kind="ExternalOutput")

    relu = mybir.ActivationFunctionType.Relu
    # First block of segment 0 is small so the first PSUM group's x arrives
    # fast; everything else uses 512-token blocks.
    segs = [(0, 0, _blocks(0, B0, first=128)), (1, B0, _blocks(B0, T))]
    NQ = _OUT_F // _WQ  # 4 weight quarters per expert
    xt_3d = None  # set inside the TileContext

    with tile.TileContext(nc) as tc, ExitStack() as ctx:
        const = ctx.enter_context(tc.tile_pool(name="const", bufs=1))
        xpool = ctx.enter_context(tc.tile_pool(name="xpool", bufs=1))
        wpool = ctx.enter_context(tc.tile_pool(name="wpool", bufs=1))
        ypool = ctx.enter_context(tc.tile_pool(name="ypool", bufs=3))
        pspool = ctx.enter_context(tc.tile_pool(name="pspool", bufs=6, space="PSUM"))

        bt = const.tile([_P, 2 * _MT], mybir.dt.float32, name="bt_sb")
        nc.sync.dma_start(bt[:], bt_d.ap())

        # [128, KT, T] view of x_t in DRAM: partition p + k-panel ko of row
        # ko*128+p; one DMA per token range moves all 8 k-panels (8 runs of
        # nb*2B per partition — large descriptors, line-rate).
        xt_3d = xt_d.ap().rearrange("(ko p) t -> p ko t", p=_P)
        w2_3d = w2_d.ap().rearrange("s (ko p) o -> s p ko o", p=_P)

        # one x tile per (segment, token-range); one w tile per (slot, quarter)
        xtiles = {}
        wtiles = {}

        def load_x(s, t0, nb):
            xt = xpool.tile([_P, _KT, nb], data_dt, name=f"x{s}_{t0}", tag=f"x{s}_{t0}")
            nc.sync.dma_start(xt[:], xt_3d[:, :, t0 : t0 + nb])
            xtiles[(s, t0)] = xt

        def load_w(s, q):
            wt = wpool.tile([_P, _KT, _WQ], data_dt, name=f"w{s}q{q}", tag=f"w{s}q{q}")
            nc.sync.dma_start(wt[:], w2_3d[s, :, :, q * _WQ : (q + 1) * _WQ])
            wtiles[(s, q)] = wt

        # Consumption-ordered loads.
        s0_blocks = segs[0][2]
        s1_blocks = segs[1][2]
        load_x(0, *s0_blocks[0])
        load_w(0, 0)
        for t0, nb in s0_blocks[1:]:
            load_x(0, t0, nb)
        for t0, nb in s1_blocks:
            load_x(1, t0, nb)
        load_w(1, 0)
        for q in range(1, NQ):
            load_w(0, q)
            load_w(1, q)

        for m in range(_MT):
            q, qc = divmod(m * _P, _WQ)
            ysb = ypool.tile([_P, T], data_dt, name="ysb", tag="ysb")
            for s, lo, blks in segs:
                wt = wtiles[(s, q)]
                ps = [
                    pspool.tile([_P, _NBLK], mybir.dt.float32, name="ps", tag="ps")[
                        :, :nb
                    ]
                    for _, nb in blks
                ]
                for k in range(_KT):
                    lhsT = wt[:, k, qc : qc + _P]
                    for bi, (t0, nb) in enumerate(blks):
                        nc.tensor.matmul(
                            ps[bi],
                            lhsT,
                            xtiles[(s, t0)][:, k, :nb],
                            start=(k == 0),
                            stop=(k == _KT - 1),
                        )
                for bi, (t0, nb) in enumerate(blks):
                    nc.scalar.activation(
                        ysb[:, t0 : t0 + nb],
                        ps[bi],
                        relu,
                        bias=bt[:, s * _MT + m : s * _MT + m + 1],
                    )
            # store the finished row block on the ScalarE DMA ring: same
            # engine as the ACTs that produced ysb, so no stall risk.
            nc.scalar.dma_start(yt_d.ap()[m * _P : (m + 1) * _P, :], ysb[:])

    nc.compile()
    _prog_cache[key] = nc
    return nc


def kernel(x, idxs, w, b):
    global LAST_RESULT
    import ml_dtypes
    from concourse.bass_utils import run_bass_kernel_spmd

    x = np.ascontiguousarray(np.asarray(x, dtype=np.float32))
    idxs = np.asarray(idxs).astype(np.int64)
    w = np.ascontiguousarray(np.asarray(w, dtype=np.float32))
    b = np.ascontiguousarray(np.asarray(b, dtype=np.float32))

    n_tok = x.shape[0]
    np_dt = {
        "float32": np.float32,
        "bfloat16": ml_dtypes.bfloat16,
        "float16": np.float16,
    }[_DTYPE]

    counts = np.bincount(idxs, minlength=_E)
    order = np.argsort(-counts, kind="stable")
    slot0 = order[:_NCORES]  # 8 largest experts
    slot1 = order[_E - 1 : _NCORES - 1 : -1]  # 8 smallest, reversed for pairing
    B0 = max(int(counts[slot0].max()), 1)
    B1 = max(int(counts[slot1].max()), 1)
    T = B0 + B1

    tok_of = [np.nonzero(idxs == e)[0] for e in range(_E)]

    in_maps = []
    metas = []
    for c in range(_NCORES):
        eA, eB = int(slot0[c]), int(slot1[c])
        ta, tb = tok_of[eA], tok_of[eB]
        xt = np.zeros((_IN_F, T), dtype=np_dt)
        xt[:, : len(ta)] = x[ta].T.astype(np_dt)
        xt[:, B0 : B0 + len(tb)] = x[tb].T.astype(np_dt)
        w2 = np.stack([w[eA], w[eB]]).astype(np_dt)
        bt = np.empty((_P, 2 * _MT), np.float32)
        bt[:, :_MT] = b[eA].reshape(_MT, _P).T
        bt[:, _MT:] = b[eB].reshape(_MT, _P).T
        in_maps.append({"xt": xt, "w2": w2, "bt": bt})
        metas.append((ta, tb))

    nc = _build_program(T, B0, _DTYPE)

    trace = os.environ.get("MOE_TRACE", "0") == "1"
    kwargs = {}
    if trace:
        kwargs["trace"] = True
        tdir = os.environ.get("MOE_TRACE_DIR")
        if tdir:
            os.makedirs(tdir, exist_ok=True)
            kwargs["tmpdir"] = tdir

    res = run_bass_kernel_spmd(nc, in_maps, core_ids=list(range(_NCORES)), **kwargs)
    LAST_RESULT = res

    y = np.empty((n_tok, _OUT_F), dtype=np.float32)
    for c in range(_NCORES):
        ta, tb = metas[c]
        yt = res.results[c]["yt"].astype(np.float32)
        y[ta] = yt[:, : len(ta)].T
        y[tb] = yt[:, B0 : B0 + len(tb)].T
    return y


# revision 9
# speedup vs baseline: 1.7683x; 1.2918x over previous
"""MoE routing kernel for Trainium2 (8 NeuronCores, Bass/Tile).

Problem: y = relu(x @ w[idxs] + b[idxs])
  x: [8192, 1024] f32, idxs: [8192] int in [0,16), w: [16, 1024, 1024] f32,
  b: [16, 1024] f32  ->  y: [8192, 1024] f32

Strategy (expert-parallel):
  - Host: bucket tokens by expert. Assign 2 experts per core (the 8 largest
    experts in slot 0, the 8 smallest in slot 1) so every core runs the SAME
    program: segment 0 = [0, B0) tokens for expert A, segment 1 = [B0, T) for
    expert B, with per-slot padding to the max expert population.
  - Host pre-transposes each core's token block to x_t [IN_F, T] (fp16) so the
    contraction dim (IN_F) lands on SBUF partitions with contiguous DMAs.
    fp16 (not bf16): same TensorE rate, ~8x lower rounding error (3.6e-4 L2).
  - Device (per core): out-feature tile m is the outer loop; for each m and
    segment, the 8 K-tile matmuls accumulate all of the segment's token
    blocks into per-block PSUM banks (weight tile stays stationary across
    blocks), then one fused ScalarE activation per block does bias-add + ReLU
    from PSUM into an fp16 row buffer [128, T]; one store per m via the
    ScalarE DMA ring (separate from the load ring, no cross-engine stall).
  - Loads are issued in consumption order (x block 0, w quarter 0 of both
    experts, remaining x, remaining w quarters) so compute starts after ~1.5MB
    instead of after the full 6.4MB.
  - Host: transpose back and scatter rows to their original token positions.
"""

import os

import numpy as np

_P = 128
_NCORES = 8
_E = 16
_IN_F = 1024
_OUT_F = 1024
_KT = _IN_F // _P  # 8 contraction tiles
_MT = _OUT_F // _P  # 8 output-feature tiles
_NBLK = 512  # token block (PSUM free dim, fp32 bank limit)
_WQ = 256  # w column-quarter width (512B rows in fp16, DMA line-rate minimum)

# Compute dtype for x/w/y on device: "float32" | "bfloat16" | "float16"
_DTYPE = os.environ.get("MOE_DTYPE", "float16")
# "raw" (hand-scheduled bass, minimal sync overhead) | "tile" (Tile framework)
_IMPL = os.environ.get("MOE_IMPL", "raw")

_prog_cache: dict = {}
LAST_RESULT = None  # BassKernelResults of the most recent device run


def _blocks(lo, hi, first=None):
    out = []
    t = lo
    if first is not None and hi - lo > first:
        out.append((t, first))
        t += first
    while t < hi:
        nb = min(_NBLK, hi - t)
        out.append((t, nb))
        t += nb
    return out


def _build_program(T: int, B0: int, dtype_str: str):
    from contextlib import ExitStack

    import concourse.mybir as mybir
    import concourse.tile as tile
    from concourse import bacc

    key = (T, B0, dtype_str)
    if key in _prog_cache:
        return _prog_cache[key]

    data_dt = {
        "float32": mybir.dt.float32,
        "bfloat16": mybir.dt.bfloat16,
        "float16": mybir.dt.float16,
    }[dtype_str]

    nc = bacc.Bacc("TRN2", target_bir_lowering=False, debug=False, num_devices=_NCORES)

    xt_d = nc.dram_tensor("xt", [_IN_F, T], data_dt, kind="ExternalInput")
    w2_d = nc.dram_tensor("w2", [2, _IN_F, _OUT_F], data_dt, kind="ExternalInput")
    bt_d = nc.dram_tensor("bt", [_P, 2 * _MT], mybir.dt.float32, kind="ExternalInput")
    yt_d = nc.dram_tensor("yt", [_OUT_F, T], data_dt, kind="ExternalOutput")

    relu = mybir.ActivationFunctionType.Relu
    # First block of segment 0 is small so the first PSUM group's x arrives
    # fast; everything else uses 512-token blocks.
    segs = [(0, 0, _blocks(0, B0, first=128)), (1, B0, _blocks(B0, T))]
    NQ = _OUT_F // _WQ  # 4 weight quarters per expert
    xt_3d = None  # set inside the TileContext

    with tile.TileContext(nc) as tc, ExitStack() as ctx:
        const = ctx.enter_context(tc.tile_pool(name="const", bufs=1))
        xpool = ctx.enter_context(tc.tile_pool(name="xpool", bufs=1))
        wpool = ctx.enter_context(tc.tile_pool(name="wpool", bufs=1))
        ypool = ctx.enter_context(tc.tile_pool(name="ypool", bufs=3))
        pspool = ctx.enter_context(tc.tile_pool(name="pspool", bufs=6, space="PSUM"))

        bt = const.tile([_P, 2 * _MT], mybir.dt.float32, name="bt_sb")
        nc.sync.dma_start(bt[:], bt_d.ap())

        # [128, KT, T] view of x_t in DRAM: partition p + k-panel ko of row
        # ko*128+p; one DMA per token range moves all 8 k-panels (8 runs of
        # nb*2B per partition — large descriptors, line-rate).
        xt_3d = xt_d.ap().rearrange("(ko p) t -> p ko t", p=_P)
        w2_3d = w2_d.ap().rearrange("s (ko p) o -> s p ko o", p=_P)

        # one x tile per (segment, token-range); one w tile per (slot, quarter)
        xtiles = {}
        wtiles = {}

        def load_x(s, t0, nb):
            xt = xpool.tile([_P, _KT, nb], data_dt, name=f"x{s}_{t0}", tag=f"x{s}_{t0}")
            nc.sync.dma_start(xt[:], xt_3d[:, :, t0 : t0 + nb])
            xtiles[(s, t0)] = xt

        def load_w(s, q):
            wt = wpool.tile([_P, _KT, _WQ], data_dt, name=f"w{s}q{q}", tag=f"w{s}q{q}")
            nc.sync.dma_start(wt[:], w2_3d[s, :, :, q * _WQ : (q + 1) * _WQ])
            wtiles[(s, q)] = wt

        # Consumption-ordered loads.
        s0_blocks = segs[0][2]
        s1_blocks = segs[1][2]
        load_x(0, *s0_blocks[0])
        load_w(0, 0)
        for t0, nb in s0_blocks[1:]:
            load_x(0, t0, nb)
        for t0, nb in s1_blocks:
            load_x(1, t0, nb)
        load_w(1, 0)
        for q in range(1, NQ):
            load_w(0, q)
            load_w(1, q)

        for m in range(_MT):
            q, qc = divmod(m * _P, _WQ)
            ysb = ypool.tile([_P, T], data_dt, name="ysb", tag="ysb")
            for s, lo, blks in segs:
                wt = wtiles[(s, q)]
                ps = [
                    pspool.tile([_P, _NBLK], mybir.dt.float32, name="ps", tag="ps")[
                        :, :nb
                    ]
                    for _, nb in blks
                ]
                for k in range(_KT):
                    lhsT = wt[:, k, qc : qc + _P]
                    for bi, (t0, nb) in enumerate(blks):
                        nc.tensor.matmul(
                            ps[bi],
                            lhsT,
                            xtiles[(s, t0)][:, k, :nb],
                            start=(k == 0),
                            stop=(k == _KT - 1),
                        )
                for bi, (t0, nb) in enumerate(blks):
                    nc.scalar.activation(
                        ysb[:, t0 : t0 + nb],
                        ps[bi],
                        relu,
                        bias=bt[:, s * _MT + m : s * _MT + m + 1],
                    )
            # store the finished row block on the ScalarE DMA ring: same
            # engine as the ACTs that produced ysb, so no stall risk.
            nc.scalar.dma_start(yt_d.ap()[m * _P : (m + 1) * _P, :], ysb[:])

    nc.compile()
    _prog_cache[key] = nc
    return nc


def _build_program_raw(T: int, B0: int, dtype_str: str):
    """Hand-scheduled bass (no Tile): explicit engine streams + semaphores.

    Engine roles:
      sync   — all input loads on the qSPDynamicHW ring, one shared dma_sem
               (+16 each, ring completes FIFO so cumulative waits suffice)
      tensor — MM stream; per PSUM-group sem inc only on the stop matmul
      scalar — bias load, fused bias+ReLU PSUM->SBUF evictions, row stores on
               its own HWDGE ring (st_sem gates row-buffer reuse)
    PSUM bank reuse is gated by act_sem (6 rotating [128,512] f32 banks).
    """
    from contextlib import ExitStack

    import concourse.bass as bass
    import concourse.mybir as mybir

    key = ("raw", T, B0, dtype_str)
    if key in _prog_cache:
        return _prog_cache[key]

    data_dt = {
        "float32": mybir.dt.float32,
        "bfloat16": mybir.dt.bfloat16,
        "float16": mybir.dt.float16,
    }[dtype_str]

    nc = bass.Bass("TRN2", target_bir_lowering=False, debug=False, num_devices=_NCORES)

    xt_d = nc.dram_tensor("xt", [_IN_F, T], data_dt, kind="ExternalInput")
    w2_d = nc.dram_tensor("w2", [2, _IN_F, _OUT_F], data_dt, kind="ExternalInput")
    bt_d = nc.dram_tensor("bt", [_P, 2 * _MT], mybir.dt.float32, kind="ExternalInput")
    yt_d = nc.dram_tensor("yt", [_OUT_F, T], data_dt, kind="ExternalOutput")

    relu = mybir.ActivationFunctionType.Relu
    segs = [(0, 0, _blocks(0, B0, first=128)), (1, B0, _blocks(B0, T))]
    flat = [(s, t0, nb) for s, _, blks in segs for t0, nb in blks]
    NQ = _OUT_F // _WQ
    NPS = 6  # rotating PSUM banks
    NY = 3  # rotating output row buffers

    xt_3d = xt_d.ap().rearrange("(ko p) t -> p ko t", p=_P)
    w2_3d = w2_d.ap().rearrange("s (ko p) o -> s p ko o", p=_P)

    # load order on the sync ring (cumulative dma_sem thresholds)
    load_order = [("x", 0)]
    load_order.append(("w", 0, 0))
    for bi in range(1, len(flat)):
        load_order.append(("x", bi))
    load_order.append(("w", 1, 0))
    for q in range(1, NQ):
        load_order.append(("w", 0, q))
        load_order.append(("w", 1, q))
    need = {k: 16 * (i + 1) for i, k in enumerate(load_order)}

    with ExitStack() as ctx:
        xsb = [
            ctx.enter_context(nc.sbuf_tensor(f"xsb{i}", [_P, _KT, nb], data_dt))
            for i, (_, _, nb) in enumerate(flat)
        ]
        wsb = [
            [
                ctx.enter_context(
                    nc.sbuf_tensor(f"wsb{s}_{q}", [_P, _KT, _WQ], data_dt)
                )
                for q in range(NQ)
            ]
            for s in range(2)
        ]
        bias_sb = ctx.enter_context(
            nc.sbuf_tensor("bias_sb", [_P, 2 * _MT], mybir.dt.float32)
        )
        ysb = [
            ctx.enter_context(nc.sbuf_tensor(f"ysb{i}", [_P, T], data_dt))
            for i in range(NY)
        ]
        ps = [
            ctx.enter_context(nc.psum_tensor(f"ps{i}", [_P, _NBLK], mybir.dt.float32))
            for i in range(NPS)
        ]
        dma_sem = ctx.enter_context(nc.semaphore("dma_sem"))
        ld2_sem = ctx.enter_context(nc.semaphore("ld2_sem"))
        mm_sem = ctx.enter_context(nc.semaphore("mm_sem"))
        act_sem = ctx.enter_context(nc.semaphore("act_sem"))
        st_sem = ctx.enter_context(nc.semaphore("st_sem"))

        # group schedule: one group per (m, seg, block); bank = gid % NPS
        groups = []
        for m in range(_MT):
            for s, _, blks in segs:
                for bi, (t0, nb) in enumerate(blks):
                    groups.append((m, s, t0, nb))

        with nc.Block() as block:

            @block.sync
            def _(sync):
                for item in load_order:
                    if item[0] == "x":
                        bi = item[1]
                        _, t0, nb = flat[bi]
                        sync.dma_start(
                            xsb[bi][:], xt_3d[:, :, t0 : t0 + nb]
                        ).then_inc(dma_sem, 16)
                    else:
                        _, s, q = item
                        sync.dma_start(
                            wsb[s][q][:], w2_3d[s, :, :, q * _WQ : (q + 1) * _WQ]
                        ).then_inc(dma_sem, 16)

            @block.tensor
            def _(tensor):
                gid = 0
                waited = 0
                for m in range(_MT):
                    q, qc = divmod(m * _P, _WQ)
                    for s, _, blks in segs:
                        L = len(blks)
                        # dma gating: weight quarter + this segment's x blocks
                        req = need[("w", s, q)]
                        for t0, nb in blks:
                            req = max(req, need[("x", flat.index((s, t0, nb)))])
                        if req > waited:
                            tensor.wait_ge(dma_sem, req)
                            waited = req
                        # bank-reuse gating against the ACT consumer
                        if gid + L > NPS:
                            tensor.wait_ge(act_sem, gid + L - NPS)
                        for k in range(_KT):
                            lhsT = wsb[s][q][:, k, qc : qc + _P]
                            for bi, (t0, nb) in enumerate(blks):
                                xi = flat.index((s, t0, nb))
                                inst = tensor.matmul(
                                    ps[(gid + bi) % NPS][:, :nb],
                                    lhsT,
                                    xsb[xi][:, k, :nb],
                                    start=(k == 0),
                                    stop=(k == _KT - 1),
                                )
                                if k == _KT - 1:
                                    inst.then_inc(mm_sem, 1)
                        gid += L

            @block.scalar
            def _(scalar):
                scalar.dma_start(bias_sb[:], bt_d.ap()).then_inc(ld2_sem, 16)
                scalar.wait_ge(ld2_sem, 16)
                gid = 0
                for m in range(_MT):
                    yb = ysb[m % NY]
                    if m >= NY:
                        scalar.wait_ge(st_sem, 16 * (m - NY + 1))
                    for s, _, blks in segs:
                        for t0, nb in blks:
                            scalar.wait_ge(mm_sem, gid + 1)
                            scalar.activation(
                                yb[:, t0 : t0 + nb],
                                ps[gid % NPS][:, :nb],
                                relu,
                                bias=bias_sb[:, s * _MT + m : s * _MT + m + 1],
                            ).then_inc(act_sem, 1)
                            gid += 1
                    scalar.dma_start(
                        yt_d.ap()[m * _P : (m + 1) * _P, :], yb[:]
                    ).then_inc(st_sem, 16)

    _prog_cache[key] = nc
    return nc


def kernel(x, idxs, w, b):
    global LAST_RESULT
    import ml_dtypes
    from concourse.bass_utils import run_bass_kernel_spmd

    x = np.ascontiguousarray(np.asarray(x, dtype=np.float32))
    idxs = np.asarray(idxs).astype(np.int64)
    w = np.ascontiguousarray(np.asarray(w, dtype=np.float32))
    b = np.ascontiguousarray(np.asarray(b, dtype=np.float32))

    n_tok = x.shape[0]
    np_dt = {
        "float32": np.float32,
        "bfloat16": ml_dtypes.bfloat16,
        "float16": np.float16,
    }[_DTYPE]

    counts = np.bincount(idxs, minlength=_E)
    order = np.argsort(-counts, kind="stable")
    slot0 = order[:_NCORES]  # 8 largest experts
    slot1 = order[_E - 1 : _NCORES - 1 : -1]  # 8 smallest, reversed for pairing
    B0 = max(int(counts[slot0].max()), 1)
    B1 = max(int(counts[slot1].max()), 1)
    T = B0 + B1

    tok_of = [np.nonzero(idxs == e)[0] for e in range(_E)]

    in_maps = []
    metas = []
    for c in range(_NCORES):
        eA, eB = int(slot0[c]), int(slot1[c])
        ta, tb = tok_of[eA], tok_of[eB]
        xt = np.zeros((_IN_F, T), dtype=np_dt)
        xt[:, : len(ta)] = x[ta].T.astype(np_dt)
        xt[:, B0 : B0 + len(tb)] = x[tb].T.astype(np_dt)
        w2 = np.stack([w[eA], w[eB]]).astype(np_dt)
        bt = np.empty((_P, 2 * _MT), np.float32)
        bt[:, :_MT] = b[eA].reshape(_MT, _P).T
        bt[:, _MT:] = b[eB].reshape(_MT, _P).T
        in_maps.append({"xt": xt, "w2": w2, "bt": bt})
        metas.append((ta, tb))

    if _IMPL == "raw":
        nc = _build_program_raw(T, B0, _DTYPE)
    else:
        nc = _build_program(T, B0, _DTYPE)

    trace = os.environ.get("MOE_TRACE", "0") == "1"
    kwargs = {}
    if trace:
        kwargs["trace"] = True
        tdir = os.environ.get("MOE_TRACE_DIR")
        if tdir:
            os.makedirs(tdir, exist_ok=True)
            kwargs["tmpdir"] = tdir

    res = run_bass_kernel_spmd(nc, in_maps, core_ids=list(range(_NCORES)), **kwargs)
    LAST_RESULT = res

    y = np.empty((n_tok, _OUT_F), dtype=np.float32)
    for c in range(_NCORES):
        ta, tb = metas[c]
        yt = res.results[c]["yt"].astype(np.float32)
        y[ta] = yt[:, : len(ta)].T
        y[tb] = yt[:, B0 : B0 + len(tb)].T
    return y
